# revision 31
# baseline (speedup 1.0000x reference)
"""Distributed Bass kernel for causal multi-head attention with RoPE.

Problem: B=2, S=2048, D=2048, H=16, HD=128 (nn_Attention_85315230368481).

Sharding: tensor-parallel over heads. Core c owns heads (2c, 2c+1); x is
replicated. Each core projects Q/K/V for its 2 heads over the full
sequence, applies RoPE, computes causal attention, and multiplies by its
row-slice of Wo, producing a partial [B*S, D] output. The host sums the
8 partials (the output is sum-sharded over head groups) — no device
collectives at all.

Performance structure:
 - Fully interleaved pipeline: per 512-row piece, K/Q/V projections for
   the piece, attention for query chunk qc==piece, then the output
   projection rows of the piece. Engine load (PE / ACT exp / DVE / GpSimd)
   stays even across the whole kernel instead of bunching per phase.
 - Causal trapezoid: per query chunk qc only key tiles kt <= 4*qc+3 are
   computed, and inside the diagonal band each matmul/exp streams only
   the live query columns. Residual triangle masking is one
   gpsimd.affine_select per diagonal tile (no mask tensor, no DVE adds).
 - Softmax denominator via an all-ones [128,128] stationary matmul that
   also broadcasts the row sums across all partitions for free; the
   divide is folded into the PSUM-evacuation scalar_tensor_tensor.
 - RoPE straight off the projection PSUM tile (PSUM-source DVE ops avoid
   the TRN2 SBUF-source slowdown): P1 = [te*c; to*c] in one full-height
   mul, P2 = [to*s; te*s] via two swapped half-height muls, then the
   two combines on GpSimd.
 - Everything bf16 into the PE (fp32 PSUM accumulate). One shared
   4-bank PSUM pool serves projection chains, score tiles and Wo chains.
 - Batches processed one at a time so x^T stays SBUF-resident per batch;
   weights load once up front; batch 1's x DMAs overlap batch 0 through
   region-level WAR release (piece by piece).
"""

import sys

import ml_dtypes
import numpy as np

if "/opt/trn_rl_repo" not in sys.path:
    sys.path.insert(0, "/opt/trn_rl_repo")

B, S, D, H = 2, 2048, 2048, 16
HD = D // H            # 128
NCORES = 8
HP = H // NCORES       # 2 heads per core
NCH = D // 128         # 16 contraction chunks
NP = S // 512          # 4 row pieces per batch (projection)
NQC = S // 512         # 4 query chunks per batch (attention)
NKT = S // 128         # 16 key tiles per batch
SCALE = 1.0 / float(np.sqrt(HD))
BF16 = ml_dtypes.bfloat16

_GRAPH = None
_TRACE = False
_LAST_EXEC_NS = None
_LAST_RES = None


def _build_graph():
    import concourse.mybir as mybir
    from concourse import bacc, tile

    f32 = mybir.dt.float32
    bf = mybir.dt.bfloat16
    Exp = mybir.ActivationFunctionType.Exp

    nc = bacc.Bacc("TRN2", target_bir_lowering=False, num_devices=NCORES)

    xT = nc.declare_dram_parameter("xT", [D, B * S], bf, isOutput=False)
    wq = nc.declare_dram_parameter("wq", [D, HP * HD], bf, isOutput=False)
    wk = nc.declare_dram_parameter("wk", [D, HP * HD], bf, isOutput=False)
    wv = nc.declare_dram_parameter("wv", [D, HP * HD], bf, isOutput=False)
    wo = nc.declare_dram_parameter("wo", [HP * HD, D], bf, isOutput=False)
    cosT = nc.declare_dram_parameter("cosT", [64, S], bf, isOutput=False)
    sinT = nc.declare_dram_parameter("sinT", [64, S], bf, isOutput=False)
    onesd = nc.declare_dram_parameter("ones", [128, 128], bf, isOutput=False)
    out = nc.declare_dram_parameter("out", [B * S, D], bf, isOutput=True)

    with nc.allow_low_precision(reason="bf16 matmul inputs; fp32 accumulate"), \
         tile.TileContext(nc) as tc:
        with (
            tc.tile_pool(name="const", bufs=1) as constp,
            tc.tile_pool(name="xpool", bufs=1) as xpool,
            tc.tile_pool(name="kqv", bufs=2) as kqvp,
            tc.tile_pool(name="wstream", bufs=3) as wpool,
            tc.tile_pool(name="wopool", bufs=1) as wopool,
            tc.tile_pool(name="ropetmp", bufs=4) as rtp,
            tc.tile_pool(name="extiles", bufs=8) as expool,
            tc.tile_pool(name="rpool", bufs=2) as rpool,
            tc.tile_pool(name="ostage", bufs=2) as ostag,
            tc.tile_pool(name="mmps", bufs=4, space="PSUM") as mmps,
            tc.tile_pool(name="attps", bufs=2, space="PSUM") as attps,
            tc.tile_pool(name="denps", bufs=2, space="PSUM") as denps,
        ):
            # cos/sin duplicated on partitions 0-63 and 64-127 so RoPE can
            # multiply the full [te; to] projection PSUM tile in one op
            wsbk = wpool.tile([128, NCH * 256], bf, tag="w", name="wsbk")
            for c in range(NCH):
                eng = nc.gpsimd if c % 2 == 0 else nc.scalar
                eng.dma_start(
                    out=wsbk[:, c * 256:(c + 1) * 256],
                    in_=wk[c * 128:(c + 1) * 128, :],
                )
            cos_sb = constp.tile([128, S], bf)
            sin_sb = constp.tile([128, S], bf)
            ones_sb = constp.tile([128, 128], bf)
            nc.scalar.dma_start(out=cos_sb[0:64, :], in_=cosT[:, :])
            nc.scalar.dma_start(out=cos_sb[64:128, :], in_=cosT[:, :])
            nc.scalar.dma_start(out=sin_sb[0:64, :], in_=sinT[:, :])
            nc.scalar.dma_start(out=sin_sb[64:128, :], in_=sinT[:, :])
            nc.scalar.dma_start(out=ones_sb[:], in_=onesd[:, :])

            # weights are batch-independent: load once, reuse for both
            # batches (wsbk first so the first projection chain can start)
            wsbq = wpool.tile([128, NCH * 256], bf, tag="w", name="wsbq")
            for c in range(NCH):
                nc.gpsimd.dma_start(
                    out=wsbq[:, c * 256:(c + 1) * 256],
                    in_=wq[c * 128:(c + 1) * 128, :],
                )
            wsbv = wpool.tile([128, NCH * 256], bf, tag="w", name="wsbv")
            for c in range(NCH):
                nc.gpsimd.dma_start(
                    out=wsbv[:, c * 256:(c + 1) * 256],
                    in_=wv[c * 128:(c + 1) * 128, :],
                )
            wosb = wopool.tile([128, HP * D], bf, tag="wo", name="wosb")
            for ht in range(HP):
                nc.gpsimd.dma_start(
                    out=wosb[:, ht * D:(ht + 1) * D],
                    in_=wo[ht * 128:(ht + 1) * 128, :],
                )

            for b in range(B):
                # Weights stream on the gpsimd queue; x tiles stream on the
                # sync queue piece-major. Everything below is emitted
                # per-512-row piece: K/Q/V projections for the piece, then
                # attention for query chunk qc=piece (keys <= diag live),
                # then the output projection rows of the piece. This keeps
                # every engine loaded evenly and lets batch 1's x DMAs
                # overlap batch 0 (region-level WAR deps release piece by
                # piece).
                xts = xpool.tile([128, NCH * S], bf, tag="xts", name="xts")
                # piece 0 via 2-chunk gather descriptors (fast start),
                # pieces 1-3 via wide per-chunk descriptors (4KB+ lines)
                for c2 in range(NCH // 2):
                    nc.sync.dma_start(
                        out=xts[:].rearrange(
                            "p (c r) -> p c r", c=NCH
                        )[:, 2 * c2:2 * c2 + 2, 0:512],
                        in_=xT[2 * c2 * 128:(2 * c2 + 2) * 128,
                               b * S:b * S + 512].rearrange(
                            "(c p) r -> p c r", c=2),
                    )
                for c in range(NCH):
                    nc.sync.dma_start(
                        out=xts[:, c * S + 512:c * S + 1024],
                        in_=xT[c * 128:(c + 1) * 128,
                               b * S + 512:b * S + 1024],
                    )
                for c in range(NCH):
                    nc.sync.dma_start(
                        out=xts[:, c * S + 1024:(c + 1) * S],
                        in_=xT[c * 128:(c + 1) * 128,
                               b * S + 1024:(b + 1) * S],
                    )

                ktsb = kqvp.tile([128, HP * S], bf, tag="kt", name="ktsb")
                qasb = kqvp.tile([128, HP * S], bf, tag="qa", name="qasb")
                vsb = kqvp.tile([128, NKT * HP * HD], bf, tag="v", name="vsb")

                for piece in range(NP):
                    po = piece * 512
                    pc = slice(po, po + 512)
                    # ---- K / Q projection + RoPE for this piece ----
                    for wsb, dst in ((wsbk, ktsb), (wsbq, qasb)):
                        for ht in range(HP):
                            pp = mmps.tile([128, 512], f32, tag="mm",
                                           name="mmtile")
                            for c in range(NCH):
                                nc.tensor.matmul(
                                    pp[:],
                                    lhsT=wsb[:, c * 256 + ht * 128:
                                             c * 256 + (ht + 1) * 128],
                                    rhs=xts[:, c * S + po:c * S + po + 512],
                                    start=(c == 0), stop=(c == NCH - 1),
                                )
                            # RoPE off the PSUM tile (PSUM-source DVE ops
                            # dodge the SBUF-source slowdown and the SBUF
                            # same-start-partition rule):
                            #   P1 = [te*c ; to*c]     (full-height mul)
                            #   P2 = [to*s ; te*s]     (two swapped halves)
                            #   re = P1.lo - P2.lo;  im = P1.hi + P2.hi
                            dc = slice(ht * S + po, ht * S + po + 512)
                            p1 = rtp.tile([128, 512], bf, tag="p1", name="p1")
                            p2 = rtp.tile([128, 512], bf, tag="p2", name="p2")
                            nc.vector.tensor_mul(p1[:], pp[:], cos_sb[:, pc])
                            nc.vector.tensor_mul(
                                p2[0:64, :], pp[64:128, :], sin_sb[64:128, pc])
                            nc.vector.tensor_mul(
                                p2[64:128, :], pp[0:64, :], sin_sb[0:64, pc])
                            nc.gpsimd.tensor_sub(
                                dst[0:64, dc], p1[0:64, :], p2[0:64, :])
                            nc.gpsimd.tensor_add(
                                dst[64:128, dc], p1[64:128, :], p2[64:128, :])

                    # ---- V projection for this piece's 4 key tiles ----
                    for rt in range(4 * piece, 4 * piece + 4):
                        vp = denps.tile([128, 512], f32, tag="den",
                                        name="dentile")
                        for c in range(NCH):
                            nc.tensor.matmul(
                                vp[:, 0:256],
                                lhsT=xts[:, c * S + rt * 128:
                                         c * S + (rt + 1) * 128],
                                rhs=wsbv[:, c * 256:(c + 1) * 256],
                                start=(c == 0), stop=(c == NCH - 1),
                            )
                        nc.vector.tensor_copy(vsb[:, rt * 256:(rt + 1) * 256],
                                              vp[:, 0:256])

                    # ---- attention for query chunk qc == piece ----
                    qc = piece
                    nkt = 4 * qc + 4          # live key tiles
                    for h in range(HP):
                        att = attps.tile([128, 512], f32, tag="att",
                                         name="atttile")
                        den = denps.tile([128, 512], f32, tag="den",
                                         name="dentile")
                        for kt in range(nkt):
                            # trapezoid: only query cols >= this key tile's
                            # start can attend; skip the fully-masked strip
                            cs = max(0, kt * 128 - qc * 512)
                            w = 512 - cs
                            sc = mmps.tile([128, 512], f32, tag="mm",
                                           name="mmtile")
                            nc.tensor.matmul(
                                sc[:, 0:w],
                                lhsT=ktsb[:, h * S + kt * 128:
                                          h * S + (kt + 1) * 128],
                                rhs=qasb[:, h * S + po + cs:
                                         h * S + po + 512],
                                start=True, stop=True,
                            )
                            ex = expool.tile([128, 512], bf, tag="ex",
                                             name="ex")
                            nc.scalar.activation(ex[:, 0:w], sc[:, 0:w], Exp,
                                                 scale=SCALE)
                            if kt >= 4 * qc:
                                # diagonal band: zero where key > query
                                nc.gpsimd.affine_select(
                                    out=ex[:, 0:w], in_=ex[:, 0:w],
                                    pattern=[[1, w]],
                                    compare_op=mybir.AluOpType.is_ge,
                                    fill=0.0,
                                    base=0,
                                    channel_multiplier=-1,
                                )
                            nc.tensor.matmul(
                                den[:, cs:512], lhsT=ones_sb[:, :],
                                rhs=ex[:, 0:w],
                                start=(kt == 0), stop=(kt == nkt - 1),
                            )
                            nc.tensor.matmul(
                                att[:, cs:512],
                                lhsT=vsb[:, kt * 256 + h * 128:
                                         kt * 256 + (h + 1) * 128],
                                rhs=ex[:, 0:w],
                                start=(kt == 0), stop=(kt == nkt - 1),
                            )
                        rsb = rpool.tile([128, 512], f32, tag="rc",
                                         name="rsb")
                        nc.vector.reciprocal_approx_fast(rsb[:], den[:])
                        nc.vector.scalar_tensor_tensor(
                            out=qasb[:, h * S + po:h * S + po + 512],
                            in0=att[:], scalar=1.0, in1=rsb[:],
                            op0=mybir.AluOpType.mult,
                            op1=mybir.AluOpType.mult,
                        )

                    # ---- output projection rows of this piece ----
                    for qt in range(4 * piece, 4 * piece + 4):
                        osb = ostag.tile([128, D], bf, tag="o", name="osb")
                        for ncol in range(4):
                            op = mmps.tile([128, 512], f32, tag="mm",
                                           name="mmtile")
                            for ht in range(HP):
                                nc.tensor.matmul(
                                    op[:],
                                    lhsT=qasb[:, ht * S + qt * 128:
                                              ht * S + (qt + 1) * 128],
                                    rhs=wosb[:, ht * D + ncol * 512:
                                             ht * D + ncol * 512 + 512],
                                    start=(ht == 0), stop=(ht == HP - 1),
                                )
                            oslice = osb[:, ncol * 512:(ncol + 1) * 512]
                            if ncol == 0:
                                nc.scalar.copy(oslice, op[:])
                            else:
                                nc.vector.tensor_copy(oslice, op[:])
                        nc.scalar.dma_start(
                            out=out[b * S + qt * 128:b * S + (qt + 1) * 128,
                                    :],
                            in_=osb[:],
                        )

    nc.compile()
    return nc


def _get_graph():
    global _GRAPH
    if _GRAPH is None:
        _GRAPH = _build_graph()
    return _GRAPH


# per-head column permutation: even dims then odd dims (RoPE partition trick)
_EVOD = np.concatenate([np.arange(0, HD, 2), np.arange(1, HD, 2)])


def kernel(x, Wq, Wk, Wv, Wo, freqs_cos, freqs_sin, mask):
    global _LAST_EXEC_NS, _LAST_RES
    from concourse.bass_utils import run_bass_kernel_spmd

    nc = _get_graph()

    x = np.asarray(x, np.float32).reshape(B * S, D)
    xTb = np.ascontiguousarray(x.T).astype(BF16)
    cos_b = np.ascontiguousarray(np.asarray(freqs_cos, np.float32).T).astype(BF16)
    sin_b = np.ascontiguousarray(np.asarray(freqs_sin, np.float32).T).astype(BF16)
    ones_b = np.ones((128, 128), BF16)
    Wq = np.asarray(Wq, np.float32)
    Wk = np.asarray(Wk, np.float32)
    Wv = np.asarray(Wv, np.float32)
    Wo = np.asarray(Wo, np.float32)

    in_maps = []
    for c in range(NCORES):
        pcols = np.concatenate([(2 * c + j) * HD + _EVOD for j in range(HP)])
        ncols = slice(2 * c * HD, (2 * c + HP) * HD)
        in_maps.append({
            "xT": xTb,
            "wq": np.ascontiguousarray(Wq[:, pcols]).astype(BF16),
            "wk": np.ascontiguousarray(Wk[:, pcols]).astype(BF16),
            "wv": np.ascontiguousarray(Wv[:, ncols]).astype(BF16),
            "wo": np.ascontiguousarray(Wo[ncols, :]).astype(BF16),
            "cosT": cos_b,
            "sinT": sin_b,
            "ones": ones_b,
        })

    res = run_bass_kernel_spmd(
        nc, in_maps, core_ids=list(range(NCORES)), trace=_TRACE,
    )
    _LAST_EXEC_NS = res.exec_time_ns
    _LAST_RES = res

    acc = np.zeros((B * S, D), np.float32)
    for c in range(NCORES):
        acc += res.results[c]["out"].astype(np.float32)
    return acc.reshape(B, S, D)


# revision 32
# speedup vs baseline: 1.0106x; 1.0106x over previous
"""Distributed Bass kernel for causal multi-head attention with RoPE.

Problem: B=2, S=2048, D=2048, H=16, HD=128 (nn_Attention_85315230368481).

Sharding: tensor-parallel over heads. Core c owns heads (2c, 2c+1); x is
replicated. Each core projects Q/K/V for its 2 heads over the full
sequence, applies RoPE, computes causal attention, and multiplies by its
row-slice of Wo, producing a partial [B*S, D] output. The host sums the
8 partials (the output is sum-sharded over head groups) — no device
collectives at all.

Performance structure:
 - Fully interleaved pipeline: per 512-row piece, K/Q/V projections for
   the piece, attention for query chunk qc==piece, then the output
   projection rows of the piece. Engine load (PE / ACT exp / DVE / GpSimd)
   stays even across the whole kernel instead of bunching per phase.
 - Causal trapezoid: per query chunk qc only key tiles kt <= 4*qc+3 are
   computed, and inside the diagonal band each matmul/exp streams only
   the live query columns. Residual triangle masking is one
   gpsimd.affine_select per diagonal tile (no mask tensor, no DVE adds).
 - Softmax denominator via an all-ones [128,128] stationary matmul that
   also broadcasts the row sums across all partitions for free; the
   divide is folded into the PSUM-evacuation scalar_tensor_tensor.
 - RoPE straight off the projection PSUM tile (PSUM-source DVE ops avoid
   the TRN2 SBUF-source slowdown): P1 = [te*c; to*c] in one full-height
   mul, P2 = [to*s; te*s] via two swapped half-height muls, then the
   two combines on GpSimd.
 - Everything bf16 into the PE (fp32 PSUM accumulate). One shared
   4-bank PSUM pool serves projection chains, score tiles and Wo chains.
 - Batches processed one at a time so x^T stays SBUF-resident per batch;
   weights load once up front; batch 1's x DMAs overlap batch 0 through
   region-level WAR release (piece by piece).
"""

import sys

import ml_dtypes
import numpy as np

if "/opt/trn_rl_repo" not in sys.path:
    sys.path.insert(0, "/opt/trn_rl_repo")

B, S, D, H = 2, 2048, 2048, 16
HD = D // H            # 128
NCORES = 8
HP = H // NCORES       # 2 heads per core
NCH = D // 128         # 16 contraction chunks
NP = S // 512          # 4 row pieces per batch (projection)
NQC = S // 512         # 4 query chunks per batch (attention)
NKT = S // 128         # 16 key tiles per batch
SCALE = 1.0 / float(np.sqrt(HD))
BF16 = ml_dtypes.bfloat16

_GRAPH = None
_TRACE = False
_LAST_EXEC_NS = None
_LAST_RES = None


def _build_graph():
    import concourse.mybir as mybir
    from concourse import bacc, tile

    f32 = mybir.dt.float32
    bf = mybir.dt.bfloat16
    Exp = mybir.ActivationFunctionType.Exp

    nc = bacc.Bacc("TRN2", target_bir_lowering=False, num_devices=NCORES)

    xT = nc.declare_dram_parameter("xT", [D, B * S], bf, isOutput=False)
    wq = nc.declare_dram_parameter("wq", [D, HP * HD], bf, isOutput=False)
    wk = nc.declare_dram_parameter("wk", [D, HP * HD], bf, isOutput=False)
    wv = nc.declare_dram_parameter("wv", [D, HP * HD], bf, isOutput=False)
    wo = nc.declare_dram_parameter("wo", [HP * HD, D], bf, isOutput=False)
    cosT = nc.declare_dram_parameter("cosT", [64, S], bf, isOutput=False)
    sinT = nc.declare_dram_parameter("sinT", [64, S], bf, isOutput=False)
    onesd = nc.declare_dram_parameter("ones", [128, 128], bf, isOutput=False)
    out = nc.declare_dram_parameter("out", [B * S, D], bf, isOutput=True)

    with nc.allow_low_precision(reason="bf16 matmul inputs; fp32 accumulate"), \
         tile.TileContext(nc) as tc:
        with (
            tc.tile_pool(name="const", bufs=1) as constp,
            tc.tile_pool(name="xpool", bufs=1) as xpool,
            tc.tile_pool(name="kqv", bufs=2) as kqvp,
            tc.tile_pool(name="wstream", bufs=3) as wpool,
            tc.tile_pool(name="wopool", bufs=1) as wopool,
            tc.tile_pool(name="ropetmp", bufs=4) as rtp,
            tc.tile_pool(name="extiles", bufs=8) as expool,
            tc.tile_pool(name="rpool", bufs=2) as rpool,
            tc.tile_pool(name="ostage", bufs=2) as ostag,
            tc.tile_pool(name="mmps", bufs=4, space="PSUM") as mmps,
            tc.tile_pool(name="attps", bufs=2, space="PSUM") as attps,
            tc.tile_pool(name="denps", bufs=2, space="PSUM") as denps,
        ):
            # cos/sin duplicated on partitions 0-63 and 64-127 so RoPE can
            # multiply the full [te; to] projection PSUM tile in one op
            wsbk = wpool.tile([128, NCH * 256], bf, tag="w", name="wsbk")
            for c in range(NCH):
                eng = nc.gpsimd if c % 2 == 0 else nc.scalar
                eng.dma_start(
                    out=wsbk[:, c * 256:(c + 1) * 256],
                    in_=wk[c * 128:(c + 1) * 128, :],
                )
            cos_sb = constp.tile([128, S], bf)
            sin_sb = constp.tile([128, S], bf)
            ones_sb = constp.tile([128, 128], bf)
            nc.scalar.dma_start(out=cos_sb[0:64, :], in_=cosT[:, :])
            nc.scalar.dma_start(out=cos_sb[64:128, :], in_=cosT[:, :])
            nc.scalar.dma_start(out=sin_sb[0:64, :], in_=sinT[:, :])
            nc.scalar.dma_start(out=sin_sb[64:128, :], in_=sinT[:, :])
            nc.scalar.dma_start(out=ones_sb[:], in_=onesd[:, :])

            # weights are batch-independent: load once, reuse for both
            # batches (wsbk first so the first projection chain can start)
            wsbq = wpool.tile([128, NCH * 256], bf, tag="w", name="wsbq")
            for c in range(NCH):
                nc.gpsimd.dma_start(
                    out=wsbq[:, c * 256:(c + 1) * 256],
                    in_=wq[c * 128:(c + 1) * 128, :],
                )
            wsbv = wpool.tile([128, NCH * 256], bf, tag="w", name="wsbv")
            for c in range(NCH):
                nc.gpsimd.dma_start(
                    out=wsbv[:, c * 256:(c + 1) * 256],
                    in_=wv[c * 128:(c + 1) * 128, :],
                )
            wosb = wopool.tile([128, HP * D], bf, tag="wo", name="wosb")
            for ht in range(HP):
                nc.gpsimd.dma_start(
                    out=wosb[:, ht * D:(ht + 1) * D],
                    in_=wo[ht * 128:(ht + 1) * 128, :],
                )

            for b in range(B):
                # Weights stream on the gpsimd queue; x tiles stream on the
                # sync queue piece-major. Everything below is emitted
                # per-512-row piece: K/Q/V projections for the piece, then
                # attention for query chunk qc=piece (keys <= diag live),
                # then the output projection rows of the piece. This keeps
                # every engine loaded evenly and lets batch 1's x DMAs
                # overlap batch 0 (region-level WAR deps release piece by
                # piece).
                xts = xpool.tile([128, NCH * S], bf, tag="xts", name="xts")
                # piece 0 via 2-chunk gather descriptors (fast start),
                # pieces 1-3 via wide per-chunk descriptors (4KB+ lines)
                for c2 in range(NCH // 2):
                    nc.sync.dma_start(
                        out=xts[:].rearrange(
                            "p (c r) -> p c r", c=NCH
                        )[:, 2 * c2:2 * c2 + 2, 0:512],
                        in_=xT[2 * c2 * 128:(2 * c2 + 2) * 128,
                               b * S:b * S + 512].rearrange(
                            "(c p) r -> p c r", c=2),
                    )
                for c in range(NCH):
                    nc.sync.dma_start(
                        out=xts[:, c * S + 512:(c + 1) * S],
                        in_=xT[c * 128:(c + 1) * 128,
                               b * S + 512:(b + 1) * S],
                    )

                ktsb = kqvp.tile([128, HP * S], bf, tag="kt", name="ktsb")
                qasb = kqvp.tile([128, HP * S], bf, tag="qa", name="qasb")
                vsb = kqvp.tile([128, NKT * HP * HD], bf, tag="v", name="vsb")

                for piece in range(NP):
                    po = piece * 512
                    pc = slice(po, po + 512)
                    # ---- K / Q projection + RoPE for this piece ----
                    for wsb, dst in ((wsbk, ktsb), (wsbq, qasb)):
                        for ht in range(HP):
                            pp = mmps.tile([128, 512], f32, tag="mm",
                                           name="mmtile")
                            for c in range(NCH):
                                nc.tensor.matmul(
                                    pp[:],
                                    lhsT=wsb[:, c * 256 + ht * 128:
                                             c * 256 + (ht + 1) * 128],
                                    rhs=xts[:, c * S + po:c * S + po + 512],
                                    start=(c == 0), stop=(c == NCH - 1),
                                )
                            # RoPE off the PSUM tile (PSUM-source DVE ops
                            # dodge the SBUF-source slowdown and the SBUF
                            # same-start-partition rule):
                            #   P1 = [te*c ; to*c]     (full-height mul)
                            #   P2 = [to*s ; te*s]     (two swapped halves)
                            #   re = P1.lo - P2.lo;  im = P1.hi + P2.hi
                            dc = slice(ht * S + po, ht * S + po + 512)
                            p1 = rtp.tile([128, 512], bf, tag="p1", name="p1")
                            p2 = rtp.tile([128, 512], bf, tag="p2", name="p2")
                            nc.vector.tensor_mul(p1[:], pp[:], cos_sb[:, pc])
                            nc.vector.tensor_mul(
                                p2[0:64, :], pp[64:128, :], sin_sb[64:128, pc])
                            nc.vector.tensor_mul(
                                p2[64:128, :], pp[0:64, :], sin_sb[0:64, pc])
                            nc.gpsimd.tensor_sub(
                                dst[0:64, dc], p1[0:64, :], p2[0:64, :])
                            nc.gpsimd.tensor_add(
                                dst[64:128, dc], p1[64:128, :], p2[64:128, :])

                    # ---- V projection for this piece's 4 key tiles ----
                    for rt in range(4 * piece, 4 * piece + 4):
                        vp = denps.tile([128, 512], f32, tag="den",
                                        name="dentile")
                        for c in range(NCH):
                            nc.tensor.matmul(
                                vp[:, 0:256],
                                lhsT=xts[:, c * S + rt * 128:
                                         c * S + (rt + 1) * 128],
                                rhs=wsbv[:, c * 256:(c + 1) * 256],
                                start=(c == 0), stop=(c == NCH - 1),
                            )
                        nc.vector.tensor_copy(vsb[:, rt * 256:(rt + 1) * 256],
                                              vp[:, 0:256])

                    # ---- attention for query chunk qc == piece ----
                    qc = piece
                    nkt = 4 * qc + 4          # live key tiles
                    for h in range(HP):
                        att = attps.tile([128, 512], f32, tag="att",
                                         name="atttile")
                        den = denps.tile([128, 512], f32, tag="den",
                                         name="dentile")
                        for kt in range(nkt):
                            # trapezoid: only query cols >= this key tile's
                            # start can attend; skip the fully-masked strip
                            cs = max(0, kt * 128 - qc * 512)
                            w = 512 - cs
                            sc = mmps.tile([128, 512], f32, tag="mm",
                                           name="mmtile")
                            nc.tensor.matmul(
                                sc[:, 0:w],
                                lhsT=ktsb[:, h * S + kt * 128:
                                          h * S + (kt + 1) * 128],
                                rhs=qasb[:, h * S + po + cs:
                                         h * S + po + 512],
                                start=True, stop=True,
                            )
                            ex = expool.tile([128, 512], bf, tag="ex",
                                             name="ex")
                            nc.scalar.activation(ex[:, 0:w], sc[:, 0:w], Exp,
                                                 scale=SCALE)
                            if kt >= 4 * qc:
                                # diagonal band: zero where key > query
                                nc.gpsimd.affine_select(
                                    out=ex[:, 0:w], in_=ex[:, 0:w],
                                    pattern=[[1, w]],
                                    compare_op=mybir.AluOpType.is_ge,
                                    fill=0.0,
                                    base=0,
                                    channel_multiplier=-1,
                                )
                            nc.tensor.matmul(
                                den[:, cs:512], lhsT=ones_sb[:, :],
                                rhs=ex[:, 0:w],
                                start=(kt == 0), stop=(kt == nkt - 1),
                            )
                            nc.tensor.matmul(
                                att[:, cs:512],
                                lhsT=vsb[:, kt * 256 + h * 128:
                                         kt * 256 + (h + 1) * 128],
                                rhs=ex[:, 0:w],
                                start=(kt == 0), stop=(kt == nkt - 1),
                            )
                        rsb = rpool.tile([128, 512], f32, tag="rc",
                                         name="rsb")
                        nc.vector.reciprocal_approx_fast(rsb[:], den[:])
                        nc.vector.scalar_tensor_tensor(
                            out=qasb[:, h * S + po:h * S + po + 512],
                            in0=att[:], scalar=1.0, in1=rsb[:],
                            op0=mybir.AluOpType.mult,
                            op1=mybir.AluOpType.mult,
                        )

                    # ---- output projection rows of this piece ----
                    for qt in range(4 * piece, 4 * piece + 4):
                        osb = ostag.tile([128, D], bf, tag="o", name="osb")
                        for ncol in range(4):
                            op = mmps.tile([128, 512], f32, tag="mm",
                                           name="mmtile")
                            for ht in range(HP):
                                nc.tensor.matmul(
                                    op[:],
                                    lhsT=qasb[:, ht * S + qt * 128:
                                              ht * S + (qt + 1) * 128],
                                    rhs=wosb[:, ht * D + ncol * 512:
                                             ht * D + ncol * 512 + 512],
                                    start=(ht == 0), stop=(ht == HP - 1),
                                )
                            oslice = osb[:, ncol * 512:(ncol + 1) * 512]
                            if ncol == 0:
                                nc.scalar.copy(oslice, op[:])
                            else:
                                nc.vector.tensor_copy(oslice, op[:])
                        nc.scalar.dma_start(
                            out=out[b * S + qt * 128:b * S + (qt + 1) * 128,
                                    :],
                            in_=osb[:],
                        )

    nc.compile()
    return nc


def _get_graph():
    global _GRAPH
    if _GRAPH is None:
        _GRAPH = _build_graph()
    return _GRAPH


# per-head column permutation: even dims then odd dims (RoPE partition trick)
_EVOD = np.concatenate([np.arange(0, HD, 2), np.arange(1, HD, 2)])


def kernel(x, Wq, Wk, Wv, Wo, freqs_cos, freqs_sin, mask):
    global _LAST_EXEC_NS, _LAST_RES
    from concourse.bass_utils import run_bass_kernel_spmd

    nc = _get_graph()

    x = np.asarray(x, np.float32).reshape(B * S, D)
    xTb = np.ascontiguousarray(x.T).astype(BF16)
    cos_b = np.ascontiguousarray(np.asarray(freqs_cos, np.float32).T).astype(BF16)
    sin_b = np.ascontiguousarray(np.asarray(freqs_sin, np.float32).T).astype(BF16)
    ones_b = np.ones((128, 128), BF16)
    Wq = np.asarray(Wq, np.float32)
    Wk = np.asarray(Wk, np.float32)
    Wv = np.asarray(Wv, np.float32)
    Wo = np.asarray(Wo, np.float32)

    in_maps = []
    for c in range(NCORES):
        pcols = np.concatenate([(2 * c + j) * HD + _EVOD for j in range(HP)])
        ncols = slice(2 * c * HD, (2 * c + HP) * HD)
        in_maps.append({
            "xT": xTb,
            "wq": np.ascontiguousarray(Wq[:, pcols]).astype(BF16),
            "wk": np.ascontiguousarray(Wk[:, pcols]).astype(BF16),
            "wv": np.ascontiguousarray(Wv[:, ncols]).astype(BF16),
            "wo": np.ascontiguousarray(Wo[ncols, :]).astype(BF16),
            "cosT": cos_b,
            "sinT": sin_b,
            "ones": ones_b,
        })

    res = run_bass_kernel_spmd(
        nc, in_maps, core_ids=list(range(NCORES)), trace=_TRACE,
    )
    _LAST_EXEC_NS = res.exec_time_ns
    _LAST_RES = res

    acc = np.zeros((B * S, D), np.float32)
    for c in range(NCORES):
        acc += res.results[c]["out"].astype(np.float32)
    return acc.reshape(B, S, D)
